# revision 8
# baseline (speedup 1.0000x reference)
"""Trainium2 Bass kernel for nn_GAT_24464133718500 (GAT + JKN-LSTM on two graphs).

Self-contained: hardcodes shapes, shards across 8 NeuronCores internally.

Strategy (per core, SPMD — no rank-dependent program):
  - Nodes row-sharded: core r owns rows [r*R, (r+1)*R), R = N/8 (host slices
    nf/adj per core, so the program itself is rank-independent).
  - All tensors kept feature-major ("transposed", [F, N]) on-chip so every
    matmul contraction sits on the partition axis; per-layer AllGather of the
    [H, R] shard rebuilds [H, N].
  - GAT attention uses the factorization
        exp(lrelu(e)) = exp(a*e) * max(1, exp((1-a)*e)),   e = s_i + s_j
    and softmax's invariance to per-row factors: any f(i) factor cancels in
    U/Z, so P[j,i] = b_j * max(1, p_i q_j) * mask^T[j,i] with
    p = exp((1-a)s_src), q = exp((1-a)s_dst), b = exp(a*s_dst), and b folded
    into the matmul's stationary operand h_b = b ⊙ [h | 1].  The inner loop is
    2 DVE ops + 1 PE matmul per (head, j-chunk) tile — no transcendentals.
  - Row-sum (Z) rides along as the appended ones-column of h_b.
  - fp16 on-chip for the O(N^2) tensors; exps pre-scaled by 2^-10 per side to
    stay in fp16 range (global scales cancel in U/Z).
"""
import sys

sys.path.insert(0, "/opt/trn_rl_repo")

import math
import numpy as np

import concourse.bass as bass
import concourse.mybir as mybir
import concourse.tile as tile
from concourse import bacc
from concourse.bass_utils import run_bass_kernel_spmd
from concourse.masks import make_identity

NCORES = 8
fp32 = mybir.dt.float32
fp16 = mybir.dt.float16
i32 = mybir.dt.int32
AF = mybir.ActivationFunctionType
OP = mybir.AluOpType

T_STEPS = 5
OUT = 64
LN2 = math.log(2.0)
EXP_BIAS = -5.0 * LN2          # per-side prescale of exp: 2^-5 each => pq scaled 2^-10
C1 = 2.0 ** -10                # floor constant in scaled space
RG = [list(range(NCORES))]

# graph configs
G1 = dict(N=3072, F=128, H=12, R=3072 // NCORES)
G2 = dict(N=2048, F=64, H=6, R=2048 // NCORES)
for g in (G1, G2):
    g["NJ"] = g["N"] // 128
    g["IC"] = g["R"] // 128


# ---------------------------------------------------------------- param layout
class ParamLayout:
    def __init__(self):
        self.offsets = {}
        self.shapes = {}
        self.size = 0

    def add(self, name, shape):
        n = int(np.prod(shape))
        self.offsets[name] = self.size
        self.shapes[name] = tuple(shape)
        self.size += n

    def slice_ap(self, par_ap, name):
        off = self.offsets[name]
        p, f = self.shapes[name]
        return par_ap[off:off + p * f].rearrange("(p f) -> p f", f=f)

    def pack(self, values):
        buf = np.zeros((self.size,), np.float32)
        for name, v in values.items():
            off = self.offsets[name]
            v = np.asarray(v, np.float32)
            assert v.shape == self.shapes[name], (name, v.shape, self.shapes[name])
            buf[off:off + v.size] = v.reshape(-1)
        return buf


def build_layout():
    L = ParamLayout()
    for gi, g in ((1, G1), (2, G2)):
        F, H = g["F"], g["H"]
        L.add(f"ln{gi}_g", (1, F))
        L.add(f"ln{gi}_b", (1, F))
        # W_aug columns: [W_flat(H*Dh) | wdst(heads) | wsrc(heads)]
        L.add(f"pre{gi}_W", (F, H + 2 * 1))
        for l in range(T_STEPS):
            L.add(f"jkn{gi}_{l}_W", (H, H + 2 * 3))
        L.add(f"out{gi}_W", (H, OUT + 2 * 1))
        L.add(f"lstm{gi}_WihT", (H, 128))
        L.add(f"lstm{gi}_WhhT", (H, 128))
        L.add(f"lstm{gi}_b", (128, 1))
    L.add("fc_WT", (2 * OUT, OUT))
    L.add("fc_b", (OUT, 1))
    return L


LAYOUT = build_layout()


def make_w_aug(p):
    """W [F,h,d], a_src/a_dst [h,d] -> [F, h*d + h + h] as [W_flat|wdst|wsrc]."""
    W = np.asarray(p["W"], np.float32)
    a_src = np.asarray(p["a_src"], np.float32)
    a_dst = np.asarray(p["a_dst"], np.float32)
    F, h, d = W.shape
    W_flat = W.reshape(F, h * d)
    wsrc = np.einsum("fhd,hd->fh", W, a_src)
    wdst = np.einsum("fhd,hd->fh", W, a_dst)
    return np.concatenate([W_flat, wdst, wsrc], axis=1)


# ---------------------------------------------------------------- device build
def _load_param16(nc, pool, par_ap, name):
    """DMA fp32 param -> SBUF, cast to fp16 tile."""
    p, f = LAYOUT.shapes[name]
    t32 = pool.tile([p, f], fp32, tag="parstage")
    nc.sync.dma_start(t32[:], LAYOUT.slice_ap(par_ap, name))
    t16 = pool.tile([p, f], fp16, tag=name)
    nc.vector.tensor_copy(t16[:], t32[:])
    return t16


def _load_param32(nc, pool, par_ap, name):
    p, f = LAYOUT.shapes[name]
    t32 = pool.tile([p, f], fp32, tag=name)
    nc.sync.dma_start(t32[:], LAYOUT.slice_ap(par_ap, name))
    return t32


def build_nc(debug_taps=False):
    nc = bacc.Bacc("TRN2", target_bir_lowering=False, debug=False,
                   enable_asserts=True, num_devices=NCORES)

    nf1 = nc.dram_tensor("nf1", [G1["R"], G1["F"]], fp32, kind="ExternalInput").ap()
    adj1 = nc.dram_tensor("adj1", [G1["R"], G1["N"]], i32, kind="ExternalInput").ap()
    nf2 = nc.dram_tensor("nf2", [G2["R"], G2["F"]], fp32, kind="ExternalInput").ap()
    adj2 = nc.dram_tensor("adj2", [G2["R"], G2["N"]], i32, kind="ExternalInput").ap()
    par = nc.dram_tensor("par", [LAYOUT.size], fp32, kind="ExternalInput").ap()
    out_d = nc.dram_tensor("out", [OUT], fp32, kind="ExternalOutput").ap()
    taps = {}

    def tap(name, shape, dtype=fp32):
        if not debug_taps:
            return None
        taps[name] = nc.dram_tensor("tap_" + name, shape, dtype,
                                    kind="ExternalOutput").ap()
        return taps[name]

    from contextlib import ExitStack
    with tile.TileContext(nc, trace_sim=False) as tc:
        with ExitStack() as _es:
            def _pool(**kw):
                return _es.enter_context(tc.tile_pool(**kw))
            constp = _pool(name="const", bufs=1)
            parp = _pool(name="par", bufs=1)
            maskp = _pool(name="mask", bufs=1)
            xfullp = _pool(name="xfull", bufs=2)
            xshp = _pool(name="xsh", bufs=8)
            workp = _pool(name="work", bufs=2)
            innerp = _pool(name="inner", bufs=4)
            hbqp = _pool(name="hbq", bufs=44)
            pBp = _pool(name="pB", bufs=7)
            adjp = _pool(name="adj", bufs=1)
            smallp = _pool(name="small", bufs=10)
            vecp = _pool(name="vec", bufs=3)
            lstmp = _pool(name="lstm", bufs=2)
            ps_hs = _pool(name="ps_hs", bufs=3, space="PSUM")
            ps_u = _pool(name="ps_u", bufs=3, space="PSUM")
            ps_m = _pool(name="ps_m", bufs=2, space="PSUM")
            dramp = _pool(name="dram", bufs=1, space="DRAM")
            ident = constp.tile([128, 128], fp32)
            make_identity(nc, ident[:])
            ones16 = constp.tile([1, 128], fp16)
            nc.vector.memset(ones16[:], 1.0)
            _cc = {}

            def const_col(val):
                if val not in _cc:
                    t = constp.tile([128, 1], fp32, tag=f"cc_{len(_cc)}")
                    nc.vector.memset(t[:], val)
                    _cc[val] = t
                return _cc[val]

            # ---------------- per-graph state built up front
            state = {}
            for gi, g, nf_ap, adj_ap in ((1, G1, nf1, adj1), (2, G2, nf2, adj2)):
                N, F, H, R, NJ, IC = g["N"], g["F"], g["H"], g["R"], g["NJ"], g["IC"]

                # --- mask prep: shard rows of adj -> transposed fp16 mask tiles
                maskT = maskp.tile([128, NJ * R], fp16, tag=f"maskT{gi}")
                CW = 1024
                for ic in range(IC):
                    for co in range(N // CW):
                        adj_i = adjp.tile([128, CW], i32, tag="adj_i")
                        nc.sync.dma_start(
                            adj_i[:],
                            adj_ap[ic * 128:(ic + 1) * 128, co * CW:(co + 1) * CW])
                        adj_f = adjp.tile([128, CW], fp32, tag="adj_f")
                        nc.vector.tensor_copy(adj_f[:], adj_i[:])
                        for j in range(CW // 128):
                            jc = co * (CW // 128) + j
                            pt = ps_m.tile([128, 128], fp32, tag="tr")
                            nc.tensor.transpose(pt[:],
                                                adj_f[:, j * 128:(j + 1) * 128],
                                                ident[:])
                            nc.vector.tensor_copy(
                                maskT[:, jc * R + ic * 128: jc * R + (ic + 1) * 128],
                                pt[:])

                # --- LayerNorm on shard rows + transpose -> xT_shard [F, R] fp16
                g_sb = _load_param32(nc, parp, par, f"ln{gi}_g")
                b_sb = _load_param32(nc, parp, par, f"ln{gi}_b")
                g16 = parp.tile([1, F], fp16, tag=f"ln{gi}_g16")
                nc.vector.tensor_copy(g16[:], g_sb[:])
                b16 = parp.tile([1, F], fp16, tag=f"ln{gi}_b16")
                nc.vector.tensor_copy(b16[:], b_sb[:])
                pgB = ps_m.tile([128, F], fp32, tag="tr")
                nc.tensor.matmul(pgB[:], ones16[:], g16[:], start=True, stop=True)
                gB = constp.tile([128, F], fp32, tag=f"gB{gi}")
                nc.vector.tensor_copy(gB[:], pgB[:])
                pbB = ps_m.tile([128, F], fp32, tag="tr")
                nc.tensor.matmul(pbB[:], ones16[:], b16[:], start=True, stop=True)
                bB = constp.tile([128, F], fp32, tag=f"bB{gi}")
                nc.vector.tensor_copy(bB[:], pbB[:])

                xT_sh = xshp.tile([F, R], fp16, tag=f"xsh{gi}")
                for ic in range(IC):
                    xn = workp.tile([128, F], fp32, tag="ln_x")
                    nc.sync.dma_start(xn[:], nf_ap[ic * 128:(ic + 1) * 128, :])
                    scr = workp.tile([128, F], fp32, tag="ln_scr")
                    ssum = smallp.tile([128, 1], fp32, tag="ln_s")
                    nc.scalar.activation(scr[:], xn[:], AF.Copy, accum_out=ssum[:])
                    sq = workp.tile([128, F], fp32, tag="ln_sq")
                    sqsum = smallp.tile([128, 1], fp32, tag="ln_sq1")
                    nc.scalar.activation(sq[:], xn[:], AF.Square, accum_out=sqsum[:])
                    mean = smallp.tile([128, 1], fp32, tag="ln_m")
                    nc.vector.tensor_scalar_mul(mean[:], ssum[:], 1.0 / F)
                    msq = smallp.tile([128, 1], fp32, tag="ln_m2")
                    nc.vector.tensor_mul(msq[:], mean[:], mean[:])
                    var = smallp.tile([128, 1], fp32, tag="ln_v")
                    nc.vector.tensor_scalar_mul(var[:], sqsum[:], 1.0 / F)
                    nc.vector.tensor_sub(var[:], var[:], msq[:])
                    std = smallp.tile([128, 1], fp32, tag="ln_std")
                    nc.scalar.activation(std[:], var[:], AF.Sqrt, bias=const_col(1e-5)[0:128, :])
                    rstd = smallp.tile([128, 1], fp32, tag="ln_r")
                    nc.vector.reciprocal(rstd[:], std[:])
                    negm = smallp.tile([128, 1], fp32, tag="ln_nm")
                    nc.vector.tensor_scalar_mul(negm[:], mean[:], -1.0)
                    xo = workp.tile([128, F], fp32, tag="ln_xo")
                    nc.vector.tensor_scalar(xo[:], xn[:], negm[:], rstd[:],
                                            OP.add, OP.mult)
                    nc.vector.tensor_mul(xo[:], xo[:], gB[:])
                    nc.vector.tensor_add(xo[:], xo[:], bB[:])
                    ptx = ps_m.tile([F, 128], fp32, tag="tr")
                    nc.tensor.transpose(ptx[:], xo[:], ident[:])
                    nc.vector.tensor_copy(xT_sh[:, ic * 128:(ic + 1) * 128], ptx[:])

                state[gi] = dict(maskT=maskT, xT_sh=xT_sh)

            # ---------------- helpers
            def allgather(gi, xsh_t, HD, R, N, tag):
                """[HD, R] fp16 shard -> [HD, N] fp16 gathered."""
                bin_ = dramp.tile([HD, R], fp16, tag=f"agin_{tag}")
                nc.sync.dma_start(bin_[:], xsh_t[:])
                bout = dramp.tile([NCORES, HD, R], fp16, addr_space="Shared",
                                  tag=f"agout_{tag}")
                nc.gpsimd.collective_compute(
                    "AllGather", OP.bypass, replica_groups=RG,
                    ins=[bin_.opt()], outs=[bout.opt()])
                xfull = xfullp.tile([HD, N], fp16, tag=f"xfull{gi}")
                for r in range(NCORES):
                    nc.sync.dma_start(xfull[:, r * R:(r + 1) * R], bout[r])
                return xfull

            def gat_layer(gi, lname, heads, dh, alpha, xT_full, xT_sh, want_gather):
                g = G1 if gi == 1 else G2
                N, R, NJ = g["N"], g["R"], g["NJ"]
                F_in = LAYOUT.shapes[lname + "_W"][0]
                HD = heads * dh
                ncols = HD + 2 * heads
                W16 = _load_param16(nc, parp, par, lname + "_W")

                # p = exp((1-a)*s_src + EXP_BIAS) for shard cols, all heads,
                # laid out on one partition: [1, heads*R]
                p_sb = vecp.tile([1, heads * R], fp16, tag="p_sb")
                for h in range(heads):
                    ps_p = ps_m.tile([1, R], fp32, tag="tr")
                    nc.tensor.matmul(ps_p[:],
                                     W16[:, HD + heads + h:HD + heads + h + 1],
                                     xT_sh[:], start=True, stop=True)
                    nc.scalar.activation(p_sb[0:1, h * R:(h + 1) * R], ps_p[:],
                                         AF.Exp,
                                         bias=const_col(EXP_BIAS)[0:1, :],
                                         scale=1.0 - alpha)
                pBs = []
                for h in range(heads):
                    ppB = ps_m.tile([128, R], fp32, tag="tr")
                    nc.tensor.matmul(ppB[:], ones16[:], p_sb[0:1, h * R:(h + 1) * R],
                                     start=True, stop=True)
                    pB = pBp.tile([128, R], fp16, tag="pB")
                    nc.vector.tensor_copy(pB[:], ppB[:])
                    pBs.append(pB)

                # A-phase per j-chunk: h, s_dst -> q, h_b
                hbs, qs = [], []
                for jc in range(NJ):
                    phs = ps_hs.tile([128, ncols], fp32, tag="hs")
                    nc.tensor.matmul(phs[:, 0:HD + heads],
                                     xT_full[:, jc * 128:(jc + 1) * 128],
                                     W16[:, 0:HD + heads], start=True, stop=True)
                    q_t = hbqp.tile([128, heads], fp32, tag="q")
                    nc.scalar.activation(q_t[:], phs[:, HD:HD + heads], AF.Exp,
                                         bias=const_col(EXP_BIAS)[0:128, :],
                                         scale=1.0 - alpha)
                    b_t = smallp.tile([128, heads], fp32, tag="b")
                    nc.scalar.activation(b_t[:], phs[:, HD:HD + heads], AF.Exp,
                                         scale=alpha)
                    hb_t = hbqp.tile([128, heads * (dh + 1)], fp16, tag="hb")
                    for h in range(heads):
                        nc.vector.tensor_copy(
                            hb_t[:, h * (dh + 1):h * (dh + 1) + 1],
                            b_t[:, h:h + 1])
                        nc.vector.tensor_scalar_mul(
                            hb_t[:, h * (dh + 1) + 1:(h + 1) * (dh + 1)],
                            phs[:, h * dh:(h + 1) * dh],
                            b_t[:, h:h + 1])
                    hbs.append(hb_t)
                    qs.append(q_t)

                # inner loop: m1 = max(pB*q, C1); P = m1*maskT; U += hb^T @ P
                maskT = state[gi]["maskT"]
                psU = ps_u.tile([32 * (heads - 1) + dh + 1, R], fp32, tag="U")
                for jc in range(NJ):
                    for h in range(heads):
                        m1 = innerp.tile([128, R], fp16, tag="m1")
                        nc.vector.tensor_scalar(m1[:], pBs[h][:],
                                                qs[jc][:, h:h + 1], C1,
                                                OP.mult, OP.max)
                        p2 = innerp.tile([128, R], fp16, tag="p2")
                        nc.vector.tensor_mul(p2[:], m1[:],
                                             maskT[:, jc * R:(jc + 1) * R])
                        nc.tensor.matmul(
                            psU[32 * h:32 * h + dh + 1, :],
                            hb_t_slice(hbs[jc], h, dh), p2[:],
                            start=(jc == 0), stop=(jc == NJ - 1))

                # finalize: Z sits on aligned row 32h (ones-first h_b).
                # Divide + ELU on the full wide tile, then DMA-extract packed.
                Wp = 32 * (heads - 1) + dh + 1
                przB = ps_m.tile([Wp, R], fp32, tag="tr")
                for h in range(heads):
                    rz = vecp.tile([1, R], fp32, tag="rz")
                    nc.vector.reciprocal(rz[:], psU[32 * h:32 * h + 1, :])
                    rz16 = vecp.tile([1, R], fp16, tag="rz16")
                    nc.vector.tensor_copy(rz16[:], rz[:])
                    nc.tensor.matmul(przB[32 * h:32 * h + dh + 1, :],
                                     ones16[:, 0:dh + 1], rz16[:],
                                     start=True, stop=True)
                rzB = workp.tile([Wp, R], fp32, tag="rzB")
                nc.vector.tensor_copy(rzB[:], przB[:])
                xw = workp.tile([Wp, R], fp32, tag="xsh_f")
                nc.vector.tensor_mul(xw[:], psU[:], rzB[:])
                # ELU (wide; junk rows harmless, discarded by extraction)
                tmin = workp.tile([Wp, R], fp32, tag="elu_min")
                nc.vector.tensor_scalar_min(tmin[:], xw[:], 0.0)
                texp = workp.tile([Wp, R], fp32, tag="elu_exp")
                nc.scalar.activation(texp[:], tmin[:], AF.Exp)
                trel = workp.tile([Wp, R], fp32, tag="elu_rel")
                nc.vector.tensor_scalar(trel[:], xw[:], 0.0, -1.0,
                                        OP.max, OP.add)
                xelu_w = workp.tile([Wp, R], fp16, tag="elu_w")
                nc.vector.tensor_add(xelu_w[:], texp[:], trel[:])
                xT_sh_next = xshp.tile([HD, R], fp16, tag=f"xsh{gi}")
                for h in range(heads):
                    nc.sync.dma_start(xT_sh_next[h * dh:(h + 1) * dh, :],
                                      xelu_w[32 * h + 1:32 * h + 1 + dh, :])
                xfull_next = None
                if want_gather:
                    xfull_next = allgather(gi, xT_sh_next, HD, R, N, lname)
                return xT_sh_next, xfull_next, xelu_w

            def hb_t_slice(hb_t, h, dh):
                return hb_t[:, h * (dh + 1):(h + 1) * (dh + 1)]

            def lstm(gi, xs_list):
                g = G1 if gi == 1 else G2
                H, R = g["H"], g["R"]
                WihT = _load_param16(nc, parp, par, f"lstm{gi}_WihT")
                WhhT = _load_param16(nc, parp, par, f"lstm{gi}_WhhT")
                bsb = _load_param32(nc, parp, par, f"lstm{gi}_b")
                h_t = None
                c_t = None
                for t in range(T_STEPS):
                    pg = ps_hs.tile([128, R], fp32, tag="hs")
                    nc.tensor.matmul(pg[:], WihT[:], xs_list[t][:],
                                     start=True, stop=(t == 0))
                    if t > 0:
                        nc.tensor.matmul(pg[:], WhhT[:], h_t[:],
                                         start=False, stop=True)
                    gi_t = lstmp.tile([H, R], fp32, tag="g_i")
                    nc.scalar.activation(gi_t[:], pg[0:H, :], AF.Sigmoid,
                                         bias=bsb[0:H, :])
                    gf_t = lstmp.tile([H, R], fp32, tag="g_f")
                    nc.scalar.activation(gf_t[:], pg[32:32 + H, :], AF.Sigmoid,
                                         bias=bsb[32:32 + H, :])
                    gg_t = lstmp.tile([H, R], fp32, tag="g_g")
                    nc.scalar.activation(gg_t[:], pg[64:64 + H, :], AF.Tanh,
                                         bias=bsb[64:64 + H, :])
                    go_t = lstmp.tile([H, R], fp32, tag="g_o")
                    nc.scalar.activation(go_t[:], pg[96:96 + H, :], AF.Sigmoid,
                                         bias=bsb[96:96 + H, :])
                    c_new = lstmp.tile([H, R], fp32, tag="c")
                    nc.vector.tensor_mul(c_new[:], gi_t[:], gg_t[:])
                    if t > 0:
                        fc_ = lstmp.tile([H, R], fp32, tag="fc")
                        nc.vector.tensor_mul(fc_[:], gf_t[:], c_t[:])
                        nc.vector.tensor_add(c_new[:], c_new[:], fc_[:])
                    tc_ = lstmp.tile([H, R], fp32, tag="tanc")
                    nc.scalar.activation(tc_[:], c_new[:], AF.Tanh)
                    h_new = lstmp.tile([H, R], fp16, tag="h")
                    nc.vector.tensor_mul(h_new[:], go_t[:], tc_[:])
                    h_t, c_t = h_new, c_new
                return h_t

            # ---------------- the network
            y_parts = {}
            lstm_in = {1: [], 2: []}
            xfull = {}
            xsh = {}
            for gi, g in ((1, G1), (2, G2)):
                xsh[gi] = state[gi]["xT_sh"]
                xfull[gi] = allgather(gi, xsh[gi], g["F"], g["R"], g["N"],
                                      f"ln{gi}")
            for gi, g in ((1, G1), (2, G2)):
                s, f, _ = gat_layer(gi, f"pre{gi}", 1, g["H"], 0.01,
                                    xfull[gi], xsh[gi], True)
                xsh[gi], xfull[gi] = s, f
            for l in range(T_STEPS):
                for gi, g in ((1, G1), (2, G2)):
                    want = l < T_STEPS - 1
                    s, f, _ = gat_layer(gi, f"jkn{gi}_{l}", 3, g["H"] // 3, 0.01,
                                        xfull[gi], xsh[gi], want)
                    lstm_in[gi].append(s)
                    if want:
                        xsh[gi], xfull[gi] = s, f
            hsh = {}
            hfull = {}
            for gi, g in ((1, G1), (2, G2)):
                hsh[gi] = lstm(gi, lstm_in[gi])
                hfull[gi] = allgather(gi, hsh[gi], g["H"], g["R"], g["N"],
                                      f"lstm{gi}")
            for gi, g in ((1, G1), (2, G2)):
                _, _, xelu_holder = gat_layer(gi, f"out{gi}", 1, OUT, 0.001,
                                              hfull[gi], hsh[gi], False)
                # xelu_holder is pre-ELU; recompute ELU'd sum:
                y_parts[gi] = xelu_holder

            # y partial = sum over shard nodes of ELU(out-layer x)
            ybounce = dramp.tile([2 * OUT], fp32, tag="ybounce")
            for gi in (1, 2):
                xelu_w = y_parts[gi]  # [65, R] fp16, rows 1..64 are ELU(x)
                Rg = G1["R"] if gi == 1 else G2["R"]
                ysum = smallp.tile([OUT + 1, 1], fp32, tag="ysum")
                yscr = workp.tile([OUT + 1, Rg], fp32, tag="yscr")
                nc.scalar.activation(yscr[:], xelu_w[:], AF.Copy,
                                     accum_out=ysum[:])
                nc.sync.dma_start(
                    ybounce[(gi - 1) * OUT:gi * OUT].rearrange("(p f) -> p f", f=1),
                    ysum[1:OUT + 1, :])
            yred = dramp.tile([2 * OUT], fp32, addr_space="Shared", tag="yred")
            nc.gpsimd.collective_compute("AllReduce", OP.add, replica_groups=RG,
                                         ins=[ybounce.opt()], outs=[yred.opt()])
            ycol = smallp.tile([2 * OUT, 1], fp32, tag="ycol")
            nc.sync.dma_start(ycol[:], yred[:].rearrange("(p f) -> p f", f=1))
            fcW = _load_param32(nc, parp, par, "fc_WT")
            fcb = _load_param32(nc, parp, par, "fc_b")
            pfc = ps_m.tile([OUT, 1], fp32, tag="tr")
            nc.tensor.matmul(pfc[:], fcW[:], ycol[:], start=True, stop=True)
            osb = smallp.tile([OUT, 1], fp32, tag="osb")
            nc.scalar.activation(osb[:], pfc[:], AF.Relu, bias=fcb[:])
            nc.sync.dma_start(out_d[:].rearrange("(p f) -> p f", f=1), osb[:])

    nc.compile()
    return nc


# ------------------------------------------------------------------- host side
_CACHE = {}


def _get_nc():
    if "nc" not in _CACHE:
        _CACHE["nc"] = build_nc()
    return _CACHE["nc"]


def make_in_maps(node_features_1, adj_mat_1, node_features_2, adj_mat_2, params):
    p = params
    vals = {}
    for gi, g in ((1, G1), (2, G2)):
        vals[f"ln{gi}_g"] = np.asarray(p[f"ln{gi}_g"], np.float32)[None, :]
        vals[f"ln{gi}_b"] = np.asarray(p[f"ln{gi}_b"], np.float32)[None, :]
        vals[f"pre{gi}_W"] = make_w_aug(p[f"pre{gi}"])
        for l in range(T_STEPS):
            vals[f"jkn{gi}_{l}_W"] = make_w_aug(p[f"jkn{gi}"])
        vals[f"out{gi}_W"] = make_w_aug(p[f"out{gi}"])
        lp = p[f"lstm{gi}"]
        H = g["H"]

        def pad_gates(WT):
            padded = np.zeros((H, 128), np.float32)
            for gate in range(4):
                padded[:, 32 * gate:32 * gate + H] = WT[:, gate * H:(gate + 1) * H]
            return padded

        vals[f"lstm{gi}_WihT"] = pad_gates(np.asarray(lp["Wih"], np.float32).T)
        vals[f"lstm{gi}_WhhT"] = pad_gates(np.asarray(lp["Whh"], np.float32).T)
        bfull = (np.asarray(lp["bih"], np.float32)
                 + np.asarray(lp["bhh"], np.float32))
        bpad = np.zeros((128, 1), np.float32)
        for gate in range(4):
            bpad[32 * gate:32 * gate + H, 0] = bfull[gate * H:(gate + 1) * H]
        vals[f"lstm{gi}_b"] = bpad
    vals["fc_WT"] = np.asarray(p["fc_W"], np.float32).T
    vals["fc_b"] = np.asarray(p["fc_b"], np.float32)[:, None]
    par = LAYOUT.pack(vals)

    nf1 = np.asarray(node_features_1, np.float32)
    nf2 = np.asarray(node_features_2, np.float32)
    a1 = np.asarray(adj_mat_1, np.int32)
    a2 = np.asarray(adj_mat_2, np.int32)
    in_maps = []
    for r in range(NCORES):
        r1 = slice(r * G1["R"], (r + 1) * G1["R"])
        r2 = slice(r * G2["R"], (r + 1) * G2["R"])
        in_maps.append(dict(
            nf1=np.ascontiguousarray(nf1[r1]),
            adj1=np.ascontiguousarray(a1[r1]),
            nf2=np.ascontiguousarray(nf2[r2]),
            adj2=np.ascontiguousarray(a2[r2]),
            par=par,
        ))
    return in_maps


def kernel(node_features_1, adj_mat_1, node_features_2, adj_mat_2, params):
    nc = _get_nc()
    in_maps = make_in_maps(node_features_1, adj_mat_1, node_features_2,
                           adj_mat_2, params)
    res = run_bass_kernel_spmd(nc, in_maps, list(range(NCORES)))
    return np.asarray(res.results[0]["out"], np.float32)
